# revision 28
# baseline (speedup 1.0000x reference)
"""DeepSeekV3 latent attention (MLA) Trainium2 Bass kernel.

Sharding: 8 cores = 2 batches x 4 head-groups (4 heads each).
Each core computes, for its (batch b, head-group hg):
  - c_kv = RMSNorm(x_b @ W_DKV.T) * w        (replicated across the 4 hg cores)
  - k_rope / q projections for its 4 heads (weights sliced on the head axis)
  - causal latent attention (no-max-sub softmax, exp/sum form)
  - out_partial = ctx_hg @ out_w[:, hg_cols].T   (row-parallel partial)
Host sums the 4 partials per batch and adds the bias.

Optimizations over the 738us baseline (trace-driven):
  - Softmax denominator no longer uses 160 ones-row matmuls: exp tiles are
    accumulated on DVE (bf16 adds) and ONE all-ones [128,128] matmul per
    context sums across partitions AND broadcasts in a single shot. The
    freed PSUM bank raises the score-bank rotation to 4.
  - Rope K tiles are zero-padded to full 128 rows (per-head tile, opposite
    half zeroed) so the rope LDWEIGHTS is a normal full-row load: the old
    64-row row_grp load could not overlap in-flight matmuls and cost
    ~200ns x 160 in double LDW serialization.
  - reciprocal_approx_fast (5x faster than reciprocal) reads the broadcast
    PSUM directly: kills the 3.4us PSUM-bank hostage + DVE FIFO clog at
    every context boundary and at the kernel tail.
  - qa for context i+1 is emitted before context i's last PV quad, so its
    PSUM waits and copies are fully covered; qa/cl drain copies alternate
    scalar/vector so neither FIFO gates the PE.
  - x / cos+sin DRAM layouts are contiguous-per-supertile (4KB descriptors)
    and x loads ride the idle sync queue: first matmul and HAM warmup come
    ~8us earlier.
  - wuk/wuvT/masks live in a whole-kernel pool loaded at the top of the
    gpsimd queue: phase B no longer waits on phase A pool teardown (which
    cost a 4.7us gap plus a HAM re-throttle at the seam).
  - Output DMA is one trigger per 128-token tile (4KB rows) instead of 4:
    the serialized trigger chain was most of the 5.7us tail.
Device layout: feature-on-partition, token-on-free throughout, so scores
come out as S^T [tk, tq] and probs feed the PV matmul with no transposes.
"""

import numpy as np
import ml_dtypes

import concourse.bass as bass
import concourse.tile as tile
from concourse import bacc
from concourse import mybir
from concourse.bass import ts
from concourse.bass_utils import run_bass_kernel_spmd

BF16 = mybir.dt.bfloat16
F32 = mybir.dt.float32
NPBF16 = ml_dtypes.bfloat16

H, HD, RD, LAT = 16, 128, 64, 512
D_IN = 2048
D_OUT = H * HD
HPC = 4  # heads per core
LC = LAT // 128
EPS = 1e-6
THETA = 10000.0
SCALE = 1.0 / float(np.sqrt(np.float32(HD + RD)))
AF = mybir.ActivationFunctionType
ALU = mybir.AluOpType


def build_mla_nc(T=2048):
    nc = bacc.Bacc("TRN2", target_bir_lowering=False)
    DC = D_IN // 128      # 16 contraction chunks for the x projections
    NT = T // 128         # 128-token tiles
    NJ = T // 512         # 512-token query supertiles

    # ---------------- I/O (all layouts are host-prepared, partition-major) ---
    xT = nc.dram_tensor("xT", [128, NJ, DC // 4, 4, 512], BF16, kind="ExternalInput")
    wdkvT = nc.dram_tensor("wdkvT", [128, DC, LAT], BF16, kind="ExternalInput")
    wkrT = nc.dram_tensor("wkrT", [128, DC, HPC * RD], BF16, kind="ExternalInput")
    wqcT = nc.dram_tensor("wqcT", [128, DC, HPC * HD], BF16, kind="ExternalInput")
    wqrT = nc.dram_tensor("wqrT", [128, DC, HPC * RD], BF16, kind="ExternalInput")
    wuk = nc.dram_tensor("wuk", [128, HPC, LAT], BF16, kind="ExternalInput")
    wuvT = nc.dram_tensor("wuvT", [128, HPC, LC, HD], BF16, kind="ExternalInput")
    owT = nc.dram_tensor("owT", [128, HPC, D_OUT], BF16, kind="ExternalInput")
    kvw = nc.dram_tensor("kvw", [128, LAT], BF16, kind="ExternalInput")
    csT = nc.dram_tensor("csT", [128, NJ, 2, 512], BF16, kind="ExternalInput")
    perm = nc.dram_tensor("perm", [128, 128], BF16, kind="ExternalInput")
    masks = nc.dram_tensor("masks", [128, 4, 512], BF16, kind="ExternalInput")
    out_p = nc.dram_tensor("out_p", [T, D_OUT], BF16, kind="ExternalOutput")

    with tile.TileContext(nc) as tc:
        with (
            tc.tile_pool(name="persist", bufs=1) as persist,
            tc.tile_pool(name="bw", bufs=1) as bw,
        ):
            # persistent activations
            ckv_nat = persist.tile([128, NT, LAT], BF16)   # [t%128, ttile, lat]
            ckvT = persist.tile([128, NT, LC, 128], BF16)  # [lat%128, ttile, lc, tok]
            # per-head rope K, zero-padded on the opposite 64-row half so the
            # attention-phase LDWEIGHTS is a normal full-row load
            kz = persist.tile([128, HPC, T], BF16)
            qTrot = persist.tile([128, 2, T], BF16)        # [pairrow, h//2, t]
            qcT = persist.tile([128, HPC, T], BF16)        # [hd, h, t]
            ctxT = persist.tile([128, HPC, T], BF16)       # [hd, h, t]

            # attention-phase weights, loaded at the very top of the gpsimd
            # queue so the A->B seam never waits on them
            wuk_s = bw.tile([128, HPC, LAT], BF16)
            wuvT_s = bw.tile([128, HPC, LC, HD], BF16)
            masks_s = bw.tile([128, 4, 512], BF16)
            ones128 = bw.tile([128, 128], BF16)

            # zero the unused rope halves once; rope writes fill the rest
            nc.vector.memset(kz[:, :, :], 0.0)
            nc.vector.memset(ones128, 1.0)

            # ============== Phase A: projections + RMSNorm + RoPE ===========
            with (
                tc.tile_pool(name="ps_a", bufs=4, space="PSUM") as ps_a,
                tc.tile_pool(name="ps_ck", bufs=1, space="PSUM") as ps_ck,
                tc.tile_pool(name="aw", bufs=1) as aw,
                tc.tile_pool(name="xs", bufs=2) as xs,
                tc.tile_pool(name="cs", bufs=2) as cs,
                tc.tile_pool(name="wka", bufs=2) as wka,
            ):
                wdkvT_s = aw.tile([128, DC, LAT], BF16)
                wkrT_s = aw.tile([128, DC, HPC * RD], BF16)
                wqcT_s = aw.tile([128, DC, HPC * HD], BF16)
                wqrT_s = aw.tile([128, DC, HPC * RD], BF16)
                # tiny-but-critical tensors first: kvw gates the c_kv norm
                # chain (and thus ckvT) at ~33us, perm gates the first rope
                perm_s = aw.tile([128, 128], BF16)
                nc.gpsimd.dma_start(perm_s, perm[:, :])
                kvw_s = aw.tile([128, LAT], BF16)
                nc.gpsimd.dma_start(kvw_s, kvw[:, :])
                # graded wdkv chunks (1,1,2,4,4,4 dc) so the dc-outer c_kv
                # stream never outruns the arriving data at startup
                for sl in (slice(0, 1), slice(1, 2), slice(2, 4),
                           slice(4, 8), slice(8, 12), slice(12, 16)):
                    nc.gpsimd.dma_start(wdkvT_s[:, sl, :], wdkvT[:, sl, :])
                # q/k projection weights ride a second queue (scalar), in
                # consumption order (wkr -> wqr -> wqc hi), so the proj
                # groups never stall on the single gpsimd queue
                for d4 in range(DC // 4):
                    sl = slice(4 * d4, 4 * d4 + 4)
                    nc.scalar.dma_start(wkrT_s[:, sl, :], wkrT[:, sl, :])
                for d4 in range(DC // 4):
                    sl = slice(4 * d4, 4 * d4 + 4)
                    nc.scalar.dma_start(wqrT_s[:, sl, :], wqrT[:, sl, :])
                for d4 in range(DC // 8):
                    sl = slice(4 * d4, 4 * d4 + 4)
                    nc.gpsimd.dma_start(wqcT_s[:, sl, :], wqcT[:, sl, :])
                for d4 in range(DC // 8, DC // 4):
                    sl = slice(4 * d4, 4 * d4 + 4)
                    nc.scalar.dma_start(wqcT_s[:, sl, :], wqcT[:, sl, :])
                eps_s = aw.tile([128, 1], F32)
                nc.vector.memset(eps_s, EPS)
                # attention weights ride the same queue, behind the A weights
                nc.gpsimd.dma_start(wuk_s, wuk[:, :, :])
                nc.gpsimd.dma_start(wuvT_s, wuvT[:, :, :, :])
                nc.gpsimd.dma_start(masks_s, masks[:, :, :])

                def rope_pair(raw, is_k, rc, jt, cos_s, sin_s, tag):
                    # raw: [128,512] sbuf with 2 heads' raw rope rows.
                    psr = ps_a.tile([128, 512], F32, tag="mm")
                    nc.tensor.matmul(psr, lhsT=perm_s, rhs=raw, start=True, stop=True)
                    tmp = wka.tile([128, 512], BF16, tag=f"{tag}_cos")
                    nc.vector.tensor_mul(tmp, raw, cos_s)
                    tmp2 = wka.tile([128, 512], BF16, tag=f"{tag}_sin")
                    nc.vector.tensor_mul(tmp2, psr, sin_s)
                    if is_k:
                        # per-head zero-padded tiles: head 2rc keeps the top
                        # 64 rows, head 2rc+1 the bottom 64 (rest stays 0)
                        nc.vector.tensor_add(
                            kz[0:64, 2 * rc, ts(jt, 512)], tmp[0:64, :], tmp2[0:64, :]
                        )
                        nc.vector.tensor_add(
                            kz[64:128, 2 * rc + 1, ts(jt, 512)],
                            tmp[64:128, :], tmp2[64:128, :],
                        )
                    else:
                        nc.vector.tensor_add(qTrot[:, rc, ts(jt, 512)], tmp, tmp2)

                rope_pending = []

                def flush_rope():
                    while rope_pending:
                        rope_pair(*rope_pending.pop(0))

                def load_x(jt, first=False):
                    xts = xs.tile([128, DC, 512], BF16, tag="x")
                    if first:
                        # fine-grained first chunks: first matmul starts as
                        # soon as dc0 lands
                        nc.sync.dma_start(xts[:, 0:1, :], xT[:, 0, 0, 0:1, :])
                        nc.sync.dma_start(xts[:, 1:2, :], xT[:, 0, 0, 1:2, :])
                        nc.sync.dma_start(xts[:, 2:4, :], xT[:, 0, 0, 2:4, :])
                    else:
                        nc.sync.dma_start(xts[:, 0:4, :], xT[:, jt, 0, :, :])
                    for q4 in range(1, 4):
                        nc.sync.dma_start(
                            xts[:, 4 * q4 : 4 * q4 + 4, :],
                            xT[:, jt, q4, :, :],
                        )
                    cs_t = cs.tile([128, 2, 512], BF16, tag="cs")
                    nc.scalar.dma_start(cs_t, csT[:, jt, :, :])
                    return xts, cs_t

                nxt = load_x(0, first=True)
                for jt in range(NJ):
                    xts, cs_t = nxt
                    if jt + 1 < NJ:
                        nxt = load_x(jt + 1)
                    cos_s = cs_t[:, 0, :]
                    sin_s = cs_t[:, 1, :]

                    # --- c_kv + RMSNorm, dc-outer so each arriving x chunk
                    #     immediately unblocks 4 matmuls (smooth startup) ---
                    ps_c = ps_ck.tile([128, 4, 512], F32, tag="ckv")
                    for dc in range(DC):
                        for tt4 in range(4):
                            nc.tensor.matmul(
                                ps_c[:, tt4, :],
                                lhsT=xts[:, dc, ts(tt4, 128)],
                                rhs=wdkvT_s[:, dc, :],
                                start=(dc == 0),
                                stop=(dc == DC - 1),
                            )
                    for tt4 in range(4):
                        tt = jt * 4 + tt4
                        # sq lives in SBUF: a PSUM sq output would make the
                        # next proj group's bank wait on the norm chain
                        sq = wka.tile([128, LAT], BF16, tag="sq")
                        ssum = wka.tile([128, 1], F32, tag="ssum")
                        nc.scalar.activation(
                            sq, ps_c[:, tt4, :], AF.Square, accum_out=ssum
                        )
                        rstd = wka.tile([128, 1], F32, tag="rstd")
                        nc.scalar.activation(
                            rstd, ssum, AF.Sqrt, bias=eps_s, scale=1.0 / LAT
                        )
                        nc.vector.reciprocal(rstd, rstd)
                        nc.vector.scalar_tensor_tensor(
                            ckv_nat[:, tt, :], ps_c[:, tt4, :], rstd, kvw_s,
                            op0=ALU.mult, op1=ALU.mult,
                        )
                        # transposed copy for the QK side (single xbar
                        # transpose per token tile; contiguous destination)
                        nc.sync.dma_start_transpose(
                            ckvT[:, tt, :, :], ckv_nat[:, tt, :]
                        )

                    # --- rope + q projections, rc0 (heads 0/1) first so the
                    #     woven attention contexts unblock as early as possible
                    def proj_group(w_s, col):
                        ps = ps_a.tile([128, 512], F32, tag="mm")
                        for dc in range(DC):
                            nc.tensor.matmul(
                                ps,
                                lhsT=w_s[:, dc, ts(col, 128)],
                                rhs=xts[:, dc, :],
                                start=(dc == 0),
                                stop=(dc == DC - 1),
                            )
                        return ps

                    for rc in range(2):
                        ps = proj_group(wkrT_s, rc)
                        raw = wka.tile([128, 512], BF16, tag="k_raw")
                        nc.scalar.copy(raw, ps)
                        rope_pending.append((raw, True, rc, jt, cos_s, sin_s, "k"))
                        ps = proj_group(wqrT_s, rc)
                        raw = wka.tile([128, 512], BF16, tag="q_raw")
                        nc.scalar.copy(raw, ps)
                        rope_pending.append((raw, False, rc, jt, cos_s, sin_s, "q"))
                        for fc in (range(2) if rc == 0 else range(2, HPC)):
                            ps = proj_group(wqcT_s, fc)
                            nc.scalar.copy(qcT[:, fc, ts(jt, 512)], ps)
                            flush_rope()
                flush_rope()

            # ============== Phase B: attention, j outer / head inner ========
            with (
                tc.tile_pool(name="ps_s", bufs=4, space="PSUM") as ps_s,
                tc.tile_pool(name="ps_pv", bufs=1, space="PSUM") as ps_pv,
                tc.tile_pool(name="bw2", bufs=1) as bw2,
                tc.tile_pool(name="qa", bufs=3) as qa_pool,
                tc.tile_pool(name="exps", bufs=12) as exps,
                tc.tile_pool(name="wkb", bufs=3) as wkb,
                tc.tile_pool(name="dnp", bufs=2) as dnp,
                tc.tile_pool(name="outs", bufs=2) as outs,
            ):
                owT_s = bw2.tile([128, HPC, D_OUT], BF16)
                for hc4 in range(HPC):
                    nc.gpsimd.dma_start(owT_s[:, hc4, :], owT[:, hc4, :])

                def emit_qa(j, h):
                    qa_t = qa_pool.tile([128, LC, 512], BF16, tag="qa")
                    for lc in range(LC):
                        ps = ps_s.tile([128, 512], F32, tag="sc")
                        nc.tensor.matmul(
                            ps,
                            lhsT=wuk_s[:, h, ts(lc, 128)],
                            rhs=qcT[:, h, ts(j, 512)],
                            start=True,
                            stop=True,
                        )
                        # alternate engines so neither FIFO gates the copies
                        if lc % 2 == 0:
                            nc.vector.tensor_copy(qa_t[:, lc, :], ps)
                        else:
                            nc.scalar.copy(qa_t[:, lc, :], ps)
                    return qa_t

                def emit_qk_quad(j, h, quad, qa_t, dn_acc):
                    # diagonal quad: key tile tq only sees query columns
                    # >= 128*tq, so every matmul shrinks to N = 512-128*tq.
                    # Earlier quads initialize the full PSUM width, so the
                    # partial-width accumulation is safe.
                    diag = quad == j
                    exs = []
                    for tq in range(4):
                        tk = 4 * quad + tq
                        c0 = 128 * tq if diag else 0
                        n = 512 - c0
                        ps = ps_s.tile([128, 512], F32, tag="sc")
                        for lc in range(LC):
                            nc.tensor.matmul(
                                ps[:, c0:512],
                                lhsT=ckvT[:, tk, lc, :],
                                rhs=qa_t[:, lc, c0:512],
                                start=(lc == 0),
                                stop=False,
                            )
                        nc.tensor.matmul(
                            ps[:, c0:512],
                            lhsT=kz[:, h, ts(tk, 128)],
                            rhs=qTrot[:, h // 2, 512 * j + c0 : 512 * (j + 1)],
                            start=False,
                            stop=True,
                        )
                        ex = exps.tile([128, 512], BF16, tag="exp")
                        nc.scalar.activation(ex[:, 0:n], ps[:, c0:512], AF.Exp, scale=SCALE)
                        if diag:
                            # only the leading 128 columns straddle the
                            # diagonal; they share one triangular mask
                            nc.vector.tensor_mul(
                                ex[:, 0:128], ex[:, 0:128], masks_s[:, 0, 0:128]
                            )
                        # softmax denominator: accumulate exp tiles on DVE
                        # (replaces a ones-row matmul per tile)
                        if quad == 0 and tq == 0:
                            nc.vector.tensor_copy(dn_acc, ex)
                        else:
                            nc.vector.tensor_add(
                                dn_acc[:, c0:512], dn_acc[:, c0:512], ex[:, 0:n]
                            )
                        exs.append((ex, c0))
                    return exs

                def emit_pv_quad(j, quad, exs, ps_ctx):
                    ntk = 4 * (j + 1)
                    for tq in range(4):
                        tk = 4 * quad + tq
                        ex, c0 = exs[tq]
                        for lc in range(LC):
                            nc.tensor.matmul(
                                ps_ctx[:, lc, c0:512],
                                lhsT=ckv_nat[:, tk, ts(lc, 128)],
                                rhs=ex[:, 0 : 512 - c0],
                                start=(tk == 0),
                                stop=(tk == ntk - 1),
                            )

                def emit_dn_bcast(dn_acc):
                    # one matmul sums the 128 partition-partials AND
                    # broadcasts the result across all partitions
                    ps_bc = ps_s.tile([128, 512], F32, tag="sc")
                    nc.tensor.matmul(
                        ps_bc, lhsT=ones128, rhs=dn_acc, start=True, stop=True
                    )
                    return ps_bc

                def emit_drain_casts(ps_ctx):
                    cl = wkb.tile([128, LC, 512], BF16, tag="ctxlat")
                    for lc in range(LC):
                        if lc % 2 == 0:
                            nc.scalar.copy(cl[:, lc, :], ps_ctx[:, lc, :])
                        else:
                            nc.vector.tensor_copy(cl[:, lc, :], ps_ctx[:, lc, :])
                    return cl

                def emit_drain_rest(j, h, cl, ps_bc):
                    db = wkb.tile([128, 512], F32, tag="db")
                    nc.vector.reciprocal_approx_fast(db, ps_bc[:, :])
                    ps_uv = ps_s.tile([128, 512], F32, tag="sc")
                    for lc in range(LC):
                        nc.tensor.matmul(
                            ps_uv,
                            lhsT=wuvT_s[:, h, lc, :],
                            rhs=cl[:, lc, :],
                            start=(lc == 0),
                            stop=(lc == LC - 1),
                        )
                    nc.vector.tensor_mul(ctxT[:, h, ts(j, 512)], ps_uv, db)

                def emit_out_proj(j):
                    for tt4 in range(4):
                        tt = 4 * j + tt4
                        ot = outs.tile([128, D_OUT], BF16, tag="ot")
                        for oc in range(D_OUT // 512):
                            ps = ps_s.tile([128, 512], F32, tag="sc")
                            for hc in range(HPC):
                                nc.tensor.matmul(
                                    ps,
                                    lhsT=ctxT[:, hc, ts(tt, 128)],
                                    rhs=owT_s[:, hc, ts(oc, 512)],
                                    start=(hc == 0),
                                    stop=(hc == HPC - 1),
                                )
                            if oc % 2 == 0:
                                nc.scalar.copy(ot[:, ts(oc, 512)], ps)
                            else:
                                nc.vector.tensor_copy(ot[:, ts(oc, 512)], ps)
                        # one DMA per 128-token tile (4KB rows): the old
                        # per-chunk triggers serialized on the sync engine
                        nc.sync.dma_start(out_p[ts(tt, 128), :], ot[:, :])

                # software-pipelined emission: drains of context i-1 ride
                # behind context i's first QK quad; qa for context i+1 is
                # emitted before context i's last PV quad.
                # j1 before j0: j0's shallow contexts then sit where j2's
                # deep quads cover their drains
                j_order = [1, 0] + list(range(2, NJ))
                contexts = [(j, h) for j in j_order for h in range(HPC)]
                pending = None       # (j, h, ps_ctx, dn_acc)
                pending_out = None
                next_qa = emit_qa(*contexts[0])
                for idx, (j, h) in enumerate(contexts):
                    qa_t = next_qa
                    next_qa = None
                    cl = None
                    if pending is not None:
                        cl = emit_drain_casts(pending[2])
                    dn_acc = dnp.tile([128, 512], BF16, tag="dn")
                    exs = emit_qk_quad(j, h, 0, qa_t, dn_acc)
                    if pending is not None:
                        ps_bc = emit_dn_bcast(pending[3])
                    if pending_out is not None:
                        emit_out_proj(pending_out)
                        pending_out = None
                    if pending is not None:
                        emit_drain_rest(pending[0], pending[1], cl, ps_bc)
                        if pending[1] == HPC - 1:
                            pending_out = pending[0]
                        pending = None
                    ps_ctx = ps_pv.tile([128, LC, 512], F32, tag="pv")
                    for quad in range(j + 1):
                        if quad > 0:
                            exs = emit_qk_quad(j, h, quad, qa_t, dn_acc)
                        if quad == j and idx + 1 < len(contexts):
                            next_qa = emit_qa(*contexts[idx + 1])
                        emit_pv_quad(j, quad, exs, ps_ctx)
                    pending = (j, h, ps_ctx, dn_acc)
                cl = emit_drain_casts(pending[2])
                ps_bc = emit_dn_bcast(pending[3])
                emit_drain_rest(pending[0], pending[1], cl, ps_bc)
                emit_out_proj(NJ - 1)

    nc.finalize()
    return nc


def _part_major(a2d):
    """[R, C] -> [128, R//128, C] with partition = R % 128."""
    r, c = a2d.shape
    return np.ascontiguousarray(
        a2d.reshape(r // 128, 128, c).transpose(1, 0, 2)
    )


def make_in_maps(x, W_DKV, kv_norm_w, W_KR, W_Q, W_UK, W_UV, out_w, offset, T):
    """Host-side sharding/layout prep. Returns the 8 per-core input dicts."""
    f32 = np.float32
    x = np.asarray(x, f32)
    W_DKV = np.asarray(W_DKV, f32)
    kv_norm_w = np.asarray(kv_norm_w, f32)
    W_KR = np.asarray(W_KR, f32)
    W_Q = np.asarray(W_Q, f32)
    W_UK = np.asarray(W_UK, f32)
    W_UV = np.asarray(W_UV, f32)
    out_w = np.asarray(out_w, f32)
    offset = int(np.asarray(offset))
    DC = D_IN // 128
    NJ = T // 512

    def bf(a):
        return np.ascontiguousarray(a).astype(NPBF16)

    # rope tables, mirroring the reference's f32 arithmetic
    inv_freq = (1.0 / (THETA ** (np.arange(0, RD, 2, dtype=f32) / f32(RD)))).astype(f32)
    pos = np.arange(offset, offset + T, dtype=f32)
    ang = (pos[:, None] * inv_freq[None, :]).astype(f32)     # [T, RD/2]
    ang = np.concatenate([ang, ang], axis=-1)                # [T, RD]
    cos_t = np.cos(ang).T                                    # [RD, T]
    sin_t = np.sin(ang).T
    cosT = np.concatenate([cos_t, cos_t], 0)                 # [128, T]
    sinT = np.concatenate([sin_t, sin_t], 0)
    # [128, NJ, 2, 512]: per-supertile contiguous cos+sin
    csT = np.stack(
        [cosT.reshape(128, NJ, 512), sinT.reshape(128, NJ, 512)], axis=2
    )

    # signed rotate-half permutation (2 heads per 128 partitions), as lhsT
    M = np.zeros((RD, RD), f32)
    for i in range(RD // 2):
        M[i, i + RD // 2] = -1.0
        M[i + RD // 2, i] = 1.0
    perm128 = np.zeros((128, 128), f32)
    perm128[:64, :64] = M
    perm128[64:, 64:] = M
    perm_lhsT = perm128.T

    # diagonal causal masks: block r masked where (128 r + p) > f
    p_idx = np.arange(128)[:, None]
    f_idx = np.arange(512)[None, :]
    masks = np.stack(
        [(128 * r + p_idx <= f_idx).astype(f32) for r in range(4)], axis=1
    )  # [128, 4, 512]

    kvw = np.broadcast_to(kv_norm_w[None, :], (128, LAT)).astype(f32)

    wuk_full = W_UK.reshape(H, HD, LAT)
    wuv_full = W_UV.reshape(H, HD, LAT)

    in_maps = []
    for b in range(2):
        xTb = _part_major(x[b].T)                            # [128, DC, T]
        # [128, NJ, DC//4, 4, 512]: per-(supertile, dc-quad) contiguous
        xTb = bf(
            xTb.reshape(128, DC // 4, 4, NJ, 512).transpose(0, 3, 1, 2, 4)
        )
        for hg in range(4):
            hs = slice(HPC * hg * HD, HPC * (hg + 1) * HD)          # content rows
            rs = slice(D_OUT + HPC * hg * RD, D_OUT + HPC * (hg + 1) * RD)
            heads = slice(HPC * hg, HPC * (hg + 1))
            wuk_c = wuk_full[heads]                                  # [4,128,512]
            wuv_c = wuv_full[heads]
            in_maps.append(
                {
                    "xT": xTb,
                    "wdkvT": bf(_part_major(W_DKV.T)),
                    "wkrT": bf(_part_major(W_KR[HPC * hg * RD : HPC * (hg + 1) * RD].T)),
                    "wqcT": bf(_part_major(W_Q[hs].T)),
                    "wqrT": bf(_part_major(W_Q[rs].T)),
                    "wuk": bf(wuk_c.transpose(1, 0, 2)),             # [128,4,512]
                    "wuvT": bf(
                        wuv_c.transpose(0, 2, 1)                     # [4,512,128]
                        .reshape(HPC, LC, 128, HD)
                        .transpose(2, 0, 1, 3)                       # [128,4,4,128]
                    ),
                    "owT": bf(
                        out_w[:, hs].T.reshape(HPC, 128, D_OUT).transpose(1, 0, 2)
                    ),
                    "kvw": bf(kvw),
                    "csT": bf(csT),
                    "perm": bf(perm_lhsT),
                    "masks": bf(masks),
                }
            )
    return in_maps


_NC_CACHE = {}


def get_nc(T=2048):
    if T not in _NC_CACHE:
        _NC_CACHE[T] = build_mla_nc(T)
    return _NC_CACHE[T]


LAST_RESULTS = None


def kernel(x, W_DKV, kv_norm_w, W_KR, W_Q, W_UK, W_UV, out_w, out_b, offset):
    global LAST_RESULTS
    import os

    x = np.asarray(x, np.float32)
    B, T, _ = x.shape
    nc = get_nc(T)
    in_maps = make_in_maps(
        x, W_DKV, kv_norm_w, W_KR, W_Q, W_UK, W_UV, out_w, offset, T
    )
    trace = os.environ.get("MLA_TRACE", "0") == "1"
    res = run_bass_kernel_spmd(
        nc, in_maps, core_ids=list(range(8)), trace=trace
    )
    LAST_RESULTS = res
    out = np.zeros((B, T, D_OUT), np.float32)
    for c, r in enumerate(res.results):
        out[c // 4] += np.asarray(r["out_p"], np.float32)
    out += np.asarray(out_b, np.float32)[None, None, :]
    return out


# revision 29
# speedup vs baseline: 1.0043x; 1.0043x over previous
"""DeepSeekV3 latent attention (MLA) Trainium2 Bass kernel.

Sharding: 8 cores = 2 batches x 4 head-groups (4 heads each).
Each core computes, for its (batch b, head-group hg):
  - c_kv = RMSNorm(x_b @ W_DKV.T) * w        (replicated across the 4 hg cores)
  - k_rope / q projections for its 4 heads (weights sliced on the head axis)
  - causal latent attention (no-max-sub softmax, exp/sum form)
  - out_partial = ctx_hg @ out_w[:, hg_cols].T   (row-parallel partial)
Host sums the 4 partials per batch and adds the bias.

Optimizations over the 738us baseline (trace-driven):
  - Softmax denominator no longer uses 160 ones-row matmuls: exp tiles are
    accumulated on DVE (bf16 adds) and ONE all-ones [128,128] matmul per
    context sums across partitions AND broadcasts in a single shot. The
    freed PSUM bank raises the score-bank rotation to 4.
  - Rope K tiles are zero-padded to full 128 rows (per-head tile, opposite
    half zeroed) so the rope LDWEIGHTS is a normal full-row load: the old
    64-row row_grp load could not overlap in-flight matmuls and cost
    ~200ns x 160 in double LDW serialization.
  - reciprocal_approx_fast (5x faster than reciprocal) reads the broadcast
    PSUM directly: kills the 3.4us PSUM-bank hostage + DVE FIFO clog at
    every context boundary and at the kernel tail.
  - qa for context i+1 is emitted before context i's last PV quad, so its
    PSUM waits and copies are fully covered; qa/cl drain copies alternate
    scalar/vector so neither FIFO gates the PE.
  - x / cos+sin DRAM layouts are contiguous-per-supertile (4KB descriptors)
    and x loads ride the idle sync queue: first matmul and HAM warmup come
    ~8us earlier.
  - wuk/wuvT/masks live in a whole-kernel pool loaded at the top of the
    gpsimd queue: phase B no longer waits on phase A pool teardown (which
    cost a 4.7us gap plus a HAM re-throttle at the seam).
  - Output DMA is one trigger per 128-token tile (4KB rows) instead of 4:
    the serialized trigger chain was most of the 5.7us tail.
Device layout: feature-on-partition, token-on-free throughout, so scores
come out as S^T [tk, tq] and probs feed the PV matmul with no transposes.
"""

import numpy as np
import ml_dtypes

import concourse.bass as bass
import concourse.tile as tile
from concourse import bacc
from concourse import mybir
from concourse.bass import ts
from concourse.bass_utils import run_bass_kernel_spmd

BF16 = mybir.dt.bfloat16
F32 = mybir.dt.float32
NPBF16 = ml_dtypes.bfloat16

H, HD, RD, LAT = 16, 128, 64, 512
D_IN = 2048
D_OUT = H * HD
HPC = 4  # heads per core
LC = LAT // 128
EPS = 1e-6
THETA = 10000.0
SCALE = 1.0 / float(np.sqrt(np.float32(HD + RD)))
AF = mybir.ActivationFunctionType
ALU = mybir.AluOpType


def build_mla_nc(T=2048):
    nc = bacc.Bacc("TRN2", target_bir_lowering=False)
    DC = D_IN // 128      # 16 contraction chunks for the x projections
    NT = T // 128         # 128-token tiles
    NJ = T // 512         # 512-token query supertiles

    # ---------------- I/O (all layouts are host-prepared, partition-major) ---
    xT = nc.dram_tensor("xT", [128, NJ, DC // 4, 4, 512], BF16, kind="ExternalInput")
    wdkvT = nc.dram_tensor("wdkvT", [128, DC, LAT], BF16, kind="ExternalInput")
    wkrT = nc.dram_tensor("wkrT", [128, DC, HPC * RD], BF16, kind="ExternalInput")
    wqcT = nc.dram_tensor("wqcT", [128, DC, HPC * HD], BF16, kind="ExternalInput")
    wqrT = nc.dram_tensor("wqrT", [128, DC, HPC * RD], BF16, kind="ExternalInput")
    wuk = nc.dram_tensor("wuk", [128, HPC, LAT], BF16, kind="ExternalInput")
    wuvT = nc.dram_tensor("wuvT", [128, HPC, LC, HD], BF16, kind="ExternalInput")
    owT = nc.dram_tensor("owT", [128, HPC, D_OUT], BF16, kind="ExternalInput")
    kvw = nc.dram_tensor("kvw", [128, LAT], BF16, kind="ExternalInput")
    csT = nc.dram_tensor("csT", [128, NJ, 2, 512], BF16, kind="ExternalInput")
    perm = nc.dram_tensor("perm", [128, 128], BF16, kind="ExternalInput")
    masks = nc.dram_tensor("masks", [128, 4, 512], BF16, kind="ExternalInput")
    out_p = nc.dram_tensor("out_p", [T, D_OUT], BF16, kind="ExternalOutput")

    with tile.TileContext(nc) as tc:
        with (
            tc.tile_pool(name="persist", bufs=1) as persist,
            tc.tile_pool(name="bw", bufs=1) as bw,
        ):
            # persistent activations
            ckv_nat = persist.tile([128, NT, LAT], BF16)   # [t%128, ttile, lat]
            ckvT = persist.tile([128, NT, LC, 128], BF16)  # [lat%128, ttile, lc, tok]
            # per-head rope K, zero-padded on the opposite 64-row half so the
            # attention-phase LDWEIGHTS is a normal full-row load
            kz = persist.tile([128, HPC, T], BF16)
            qTrot = persist.tile([128, 2, T], BF16)        # [pairrow, h//2, t]
            qcT = persist.tile([128, HPC, T], BF16)        # [hd, h, t]
            ctxT = persist.tile([128, HPC, T], BF16)       # [hd, h, t]

            # attention-phase weights, loaded at the very top of the gpsimd
            # queue so the A->B seam never waits on them
            wuk_s = bw.tile([128, HPC, LAT], BF16)
            wuvT_s = bw.tile([128, HPC, LC, HD], BF16)
            masks_s = bw.tile([128, 4, 512], BF16)
            ones128 = bw.tile([128, 128], BF16)

            # zero the unused rope halves once; rope writes fill the rest
            nc.vector.memset(kz[:, :, :], 0.0)
            nc.vector.memset(ones128, 1.0)

            # ============== Phase A: projections + RMSNorm + RoPE ===========
            with (
                tc.tile_pool(name="ps_a", bufs=4, space="PSUM") as ps_a,
                tc.tile_pool(name="ps_ck", bufs=1, space="PSUM") as ps_ck,
                tc.tile_pool(name="aw", bufs=1) as aw,
                tc.tile_pool(name="xs", bufs=2) as xs,
                tc.tile_pool(name="cs", bufs=2) as cs,
                tc.tile_pool(name="wka", bufs=2) as wka,
            ):
                wdkvT_s = aw.tile([128, DC, LAT], BF16)
                wkrT_s = aw.tile([128, DC, HPC * RD], BF16)
                wqcT_s = aw.tile([128, DC, HPC * HD], BF16)
                wqrT_s = aw.tile([128, DC, HPC * RD], BF16)
                # tiny-but-critical tensors first: kvw gates the c_kv norm
                # chain (and thus ckvT) at ~33us, perm gates the first rope
                perm_s = aw.tile([128, 128], BF16)
                nc.gpsimd.dma_start(perm_s, perm[:, :])
                kvw_s = aw.tile([128, LAT], BF16)
                nc.gpsimd.dma_start(kvw_s, kvw[:, :])
                # first wdkv chunk is tiny so the first matmul starts early
                nc.gpsimd.dma_start(wdkvT_s[:, 0:1, :], wdkvT[:, 0:1, :])
                nc.gpsimd.dma_start(wdkvT_s[:, 1:4, :], wdkvT[:, 1:4, :])
                for d4 in range(1, DC // 4):
                    sl = slice(4 * d4, 4 * d4 + 4)
                    nc.gpsimd.dma_start(wdkvT_s[:, sl, :], wdkvT[:, sl, :])
                # q/k projection weights ride a second queue (scalar), in
                # consumption order (wkr -> wqr), so the proj groups never
                # stall on the single gpsimd queue
                for d4 in range(DC // 4):
                    sl = slice(4 * d4, 4 * d4 + 4)
                    nc.scalar.dma_start(wkrT_s[:, sl, :], wkrT[:, sl, :])
                for d4 in range(DC // 4):
                    sl = slice(4 * d4, 4 * d4 + 4)
                    nc.scalar.dma_start(wqrT_s[:, sl, :], wqrT[:, sl, :])
                for d4 in range(DC // 4):
                    sl = slice(4 * d4, 4 * d4 + 4)
                    nc.gpsimd.dma_start(wqcT_s[:, sl, :], wqcT[:, sl, :])
                eps_s = aw.tile([128, 1], F32)
                nc.vector.memset(eps_s, EPS)
                # attention weights ride the same queue, behind the A weights
                nc.gpsimd.dma_start(wuk_s, wuk[:, :, :])
                nc.gpsimd.dma_start(wuvT_s, wuvT[:, :, :, :])
                nc.gpsimd.dma_start(masks_s, masks[:, :, :])

                def rope_pair(raw, is_k, rc, jt, cos_s, sin_s, tag):
                    # raw: [128,512] sbuf with 2 heads' raw rope rows.
                    psr = ps_a.tile([128, 512], F32, tag="mm")
                    nc.tensor.matmul(psr, lhsT=perm_s, rhs=raw, start=True, stop=True)
                    tmp = wka.tile([128, 512], BF16, tag=f"{tag}_cos")
                    nc.vector.tensor_mul(tmp, raw, cos_s)
                    tmp2 = wka.tile([128, 512], BF16, tag=f"{tag}_sin")
                    nc.vector.tensor_mul(tmp2, psr, sin_s)
                    if is_k:
                        # per-head zero-padded tiles: head 2rc keeps the top
                        # 64 rows, head 2rc+1 the bottom 64 (rest stays 0)
                        nc.vector.tensor_add(
                            kz[0:64, 2 * rc, ts(jt, 512)], tmp[0:64, :], tmp2[0:64, :]
                        )
                        nc.vector.tensor_add(
                            kz[64:128, 2 * rc + 1, ts(jt, 512)],
                            tmp[64:128, :], tmp2[64:128, :],
                        )
                    else:
                        nc.vector.tensor_add(qTrot[:, rc, ts(jt, 512)], tmp, tmp2)

                rope_pending = []

                def flush_rope():
                    while rope_pending:
                        rope_pair(*rope_pending.pop(0))

                def load_x(jt, first=False):
                    xts = xs.tile([128, DC, 512], BF16, tag="x")
                    if first:
                        # fine-grained first chunks: first matmul starts as
                        # soon as dc0 lands
                        nc.sync.dma_start(xts[:, 0:1, :], xT[:, 0, 0, 0:1, :])
                        nc.sync.dma_start(xts[:, 1:4, :], xT[:, 0, 0, 1:4, :])
                    else:
                        nc.sync.dma_start(xts[:, 0:4, :], xT[:, jt, 0, :, :])
                    for q4 in range(1, 4):
                        nc.sync.dma_start(
                            xts[:, 4 * q4 : 4 * q4 + 4, :],
                            xT[:, jt, q4, :, :],
                        )
                    cs_t = cs.tile([128, 2, 512], BF16, tag="cs")
                    nc.scalar.dma_start(cs_t, csT[:, jt, :, :])
                    return xts, cs_t

                nxt = load_x(0, first=True)
                for jt in range(NJ):
                    xts, cs_t = nxt
                    if jt + 1 < NJ:
                        nxt = load_x(jt + 1)
                    cos_s = cs_t[:, 0, :]
                    sin_s = cs_t[:, 1, :]

                    # --- c_kv + RMSNorm, dc-outer so each arriving x chunk
                    #     immediately unblocks 4 matmuls (smooth startup) ---
                    ps_c = ps_ck.tile([128, 4, 512], F32, tag="ckv")
                    for dc in range(DC):
                        for tt4 in range(4):
                            nc.tensor.matmul(
                                ps_c[:, tt4, :],
                                lhsT=xts[:, dc, ts(tt4, 128)],
                                rhs=wdkvT_s[:, dc, :],
                                start=(dc == 0),
                                stop=(dc == DC - 1),
                            )
                    for tt4 in range(4):
                        tt = jt * 4 + tt4
                        # sq lives in SBUF: a PSUM sq output would make the
                        # next proj group's bank wait on the norm chain
                        sq = wka.tile([128, LAT], BF16, tag="sq")
                        ssum = wka.tile([128, 1], F32, tag="ssum")
                        nc.scalar.activation(
                            sq, ps_c[:, tt4, :], AF.Square, accum_out=ssum
                        )
                        rstd = wka.tile([128, 1], F32, tag="rstd")
                        nc.scalar.activation(
                            rstd, ssum, AF.Sqrt, bias=eps_s, scale=1.0 / LAT
                        )
                        nc.vector.reciprocal(rstd, rstd)
                        nc.vector.scalar_tensor_tensor(
                            ckv_nat[:, tt, :], ps_c[:, tt4, :], rstd, kvw_s,
                            op0=ALU.mult, op1=ALU.mult,
                        )
                        # transposed copy for the QK side (single xbar
                        # transpose per token tile; contiguous destination)
                        nc.sync.dma_start_transpose(
                            ckvT[:, tt, :, :], ckv_nat[:, tt, :]
                        )

                    # --- rope + q projections, rc0 (heads 0/1) first so the
                    #     woven attention contexts unblock as early as possible
                    def proj_group(w_s, col):
                        ps = ps_a.tile([128, 512], F32, tag="mm")
                        for dc in range(DC):
                            nc.tensor.matmul(
                                ps,
                                lhsT=w_s[:, dc, ts(col, 128)],
                                rhs=xts[:, dc, :],
                                start=(dc == 0),
                                stop=(dc == DC - 1),
                            )
                        return ps

                    for rc in range(2):
                        ps = proj_group(wkrT_s, rc)
                        raw = wka.tile([128, 512], BF16, tag="k_raw")
                        nc.scalar.copy(raw, ps)
                        rope_pending.append((raw, True, rc, jt, cos_s, sin_s, "k"))
                        ps = proj_group(wqrT_s, rc)
                        raw = wka.tile([128, 512], BF16, tag="q_raw")
                        nc.scalar.copy(raw, ps)
                        rope_pending.append((raw, False, rc, jt, cos_s, sin_s, "q"))
                        for fc in (range(2) if rc == 0 else range(2, HPC)):
                            ps = proj_group(wqcT_s, fc)
                            nc.scalar.copy(qcT[:, fc, ts(jt, 512)], ps)
                            flush_rope()
                flush_rope()

            # ============== Phase B: attention, j outer / head inner ========
            with (
                tc.tile_pool(name="ps_s", bufs=4, space="PSUM") as ps_s,
                tc.tile_pool(name="ps_pv", bufs=1, space="PSUM") as ps_pv,
                tc.tile_pool(name="bw2", bufs=1) as bw2,
                tc.tile_pool(name="qa", bufs=3) as qa_pool,
                tc.tile_pool(name="exps", bufs=12) as exps,
                tc.tile_pool(name="wkb", bufs=3) as wkb,
                tc.tile_pool(name="dnp", bufs=2) as dnp,
                tc.tile_pool(name="outs", bufs=2) as outs,
            ):
                owT_s = bw2.tile([128, HPC, D_OUT], BF16)
                for hc4 in range(HPC):
                    nc.gpsimd.dma_start(owT_s[:, hc4, :], owT[:, hc4, :])

                def emit_qa(j, h):
                    qa_t = qa_pool.tile([128, LC, 512], BF16, tag="qa")
                    for lc in range(LC):
                        ps = ps_s.tile([128, 512], F32, tag="sc")
                        nc.tensor.matmul(
                            ps,
                            lhsT=wuk_s[:, h, ts(lc, 128)],
                            rhs=qcT[:, h, ts(j, 512)],
                            start=True,
                            stop=True,
                        )
                        # alternate engines so neither FIFO gates the copies
                        if lc % 2 == 0:
                            nc.vector.tensor_copy(qa_t[:, lc, :], ps)
                        else:
                            nc.scalar.copy(qa_t[:, lc, :], ps)
                    return qa_t

                def emit_qk_quad(j, h, quad, qa_t, dn_acc):
                    # diagonal quad: key tile tq only sees query columns
                    # >= 128*tq, so every matmul shrinks to N = 512-128*tq.
                    # Earlier quads initialize the full PSUM width, so the
                    # partial-width accumulation is safe.
                    diag = quad == j
                    exs = []
                    for tq in range(4):
                        tk = 4 * quad + tq
                        c0 = 128 * tq if diag else 0
                        n = 512 - c0
                        ps = ps_s.tile([128, 512], F32, tag="sc")
                        for lc in range(LC):
                            nc.tensor.matmul(
                                ps[:, c0:512],
                                lhsT=ckvT[:, tk, lc, :],
                                rhs=qa_t[:, lc, c0:512],
                                start=(lc == 0),
                                stop=False,
                            )
                        nc.tensor.matmul(
                            ps[:, c0:512],
                            lhsT=kz[:, h, ts(tk, 128)],
                            rhs=qTrot[:, h // 2, 512 * j + c0 : 512 * (j + 1)],
                            start=False,
                            stop=True,
                        )
                        ex = exps.tile([128, 512], BF16, tag="exp")
                        nc.scalar.activation(ex[:, 0:n], ps[:, c0:512], AF.Exp, scale=SCALE)
                        if diag:
                            # only the leading 128 columns straddle the
                            # diagonal; they share one triangular mask
                            nc.vector.tensor_mul(
                                ex[:, 0:128], ex[:, 0:128], masks_s[:, 0, 0:128]
                            )
                        # softmax denominator: accumulate exp tiles on DVE
                        # (replaces a ones-row matmul per tile)
                        if quad == 0 and tq == 0:
                            nc.vector.tensor_copy(dn_acc, ex)
                        else:
                            nc.vector.tensor_add(
                                dn_acc[:, c0:512], dn_acc[:, c0:512], ex[:, 0:n]
                            )
                        exs.append((ex, c0))
                    return exs

                def emit_pv_quad(j, quad, exs, ps_ctx):
                    ntk = 4 * (j + 1)
                    for tq in range(4):
                        tk = 4 * quad + tq
                        ex, c0 = exs[tq]
                        for lc in range(LC):
                            nc.tensor.matmul(
                                ps_ctx[:, lc, c0:512],
                                lhsT=ckv_nat[:, tk, ts(lc, 128)],
                                rhs=ex[:, 0 : 512 - c0],
                                start=(tk == 0),
                                stop=(tk == ntk - 1),
                            )

                def emit_dn_bcast(dn_acc):
                    # one matmul sums the 128 partition-partials AND
                    # broadcasts the result across all partitions
                    ps_bc = ps_s.tile([128, 512], F32, tag="sc")
                    nc.tensor.matmul(
                        ps_bc, lhsT=ones128, rhs=dn_acc, start=True, stop=True
                    )
                    return ps_bc

                def emit_drain_casts(ps_ctx):
                    cl = wkb.tile([128, LC, 512], BF16, tag="ctxlat")
                    for lc in range(LC):
                        if lc % 2 == 0:
                            nc.scalar.copy(cl[:, lc, :], ps_ctx[:, lc, :])
                        else:
                            nc.vector.tensor_copy(cl[:, lc, :], ps_ctx[:, lc, :])
                    return cl

                def emit_drain_rest(j, h, cl, ps_bc):
                    db = wkb.tile([128, 512], F32, tag="db")
                    nc.vector.reciprocal_approx_fast(db, ps_bc[:, :])
                    ps_uv = ps_s.tile([128, 512], F32, tag="sc")
                    for lc in range(LC):
                        nc.tensor.matmul(
                            ps_uv,
                            lhsT=wuvT_s[:, h, lc, :],
                            rhs=cl[:, lc, :],
                            start=(lc == 0),
                            stop=(lc == LC - 1),
                        )
                    nc.vector.tensor_mul(ctxT[:, h, ts(j, 512)], ps_uv, db)

                def emit_out_proj(j):
                    for tt4 in range(4):
                        tt = 4 * j + tt4
                        ot = outs.tile([128, D_OUT], BF16, tag="ot")
                        for oc in range(D_OUT // 512):
                            ps = ps_s.tile([128, 512], F32, tag="sc")
                            for hc in range(HPC):
                                nc.tensor.matmul(
                                    ps,
                                    lhsT=ctxT[:, hc, ts(tt, 128)],
                                    rhs=owT_s[:, hc, ts(oc, 512)],
                                    start=(hc == 0),
                                    stop=(hc == HPC - 1),
                                )
                            if oc % 2 == 0:
                                nc.scalar.copy(ot[:, ts(oc, 512)], ps)
                            else:
                                nc.vector.tensor_copy(ot[:, ts(oc, 512)], ps)
                        # one DMA per 128-token tile (4KB rows): the old
                        # per-chunk triggers serialized on the sync engine
                        nc.sync.dma_start(out_p[ts(tt, 128), :], ot[:, :])

                # software-pipelined emission: drains of context i-1 ride
                # behind context i's first QK quad; qa for context i+1 is
                # emitted before context i's last PV quad.
                # j1 before j0: j0's shallow contexts then sit where j2's
                # deep quads cover their drains
                j_order = [1, 0] + list(range(2, NJ))
                contexts = [(j, h) for j in j_order for h in range(HPC)]
                pending = None       # (j, h, ps_ctx, dn_acc)
                pending_out = None
                next_qa = emit_qa(*contexts[0])
                for idx, (j, h) in enumerate(contexts):
                    qa_t = next_qa
                    next_qa = None
                    cl = None
                    if pending is not None:
                        cl = emit_drain_casts(pending[2])
                    dn_acc = dnp.tile([128, 512], BF16, tag="dn")
                    exs = emit_qk_quad(j, h, 0, qa_t, dn_acc)
                    if pending is not None:
                        ps_bc = emit_dn_bcast(pending[3])
                    if pending_out is not None:
                        emit_out_proj(pending_out)
                        pending_out = None
                    if pending is not None:
                        emit_drain_rest(pending[0], pending[1], cl, ps_bc)
                        if pending[1] == HPC - 1:
                            pending_out = pending[0]
                        pending = None
                    ps_ctx = ps_pv.tile([128, LC, 512], F32, tag="pv")
                    for quad in range(j + 1):
                        if quad > 0:
                            exs = emit_qk_quad(j, h, quad, qa_t, dn_acc)
                        if quad == j and idx + 1 < len(contexts):
                            next_qa = emit_qa(*contexts[idx + 1])
                        emit_pv_quad(j, quad, exs, ps_ctx)
                    pending = (j, h, ps_ctx, dn_acc)
                cl = emit_drain_casts(pending[2])
                ps_bc = emit_dn_bcast(pending[3])
                emit_drain_rest(pending[0], pending[1], cl, ps_bc)
                emit_out_proj(NJ - 1)

    nc.finalize()
    return nc


def _part_major(a2d):
    """[R, C] -> [128, R//128, C] with partition = R % 128."""
    r, c = a2d.shape
    return np.ascontiguousarray(
        a2d.reshape(r // 128, 128, c).transpose(1, 0, 2)
    )


def make_in_maps(x, W_DKV, kv_norm_w, W_KR, W_Q, W_UK, W_UV, out_w, offset, T):
    """Host-side sharding/layout prep. Returns the 8 per-core input dicts."""
    f32 = np.float32
    x = np.asarray(x, f32)
    W_DKV = np.asarray(W_DKV, f32)
    kv_norm_w = np.asarray(kv_norm_w, f32)
    W_KR = np.asarray(W_KR, f32)
    W_Q = np.asarray(W_Q, f32)
    W_UK = np.asarray(W_UK, f32)
    W_UV = np.asarray(W_UV, f32)
    out_w = np.asarray(out_w, f32)
    offset = int(np.asarray(offset))
    DC = D_IN // 128
    NJ = T // 512

    def bf(a):
        return np.ascontiguousarray(a).astype(NPBF16)

    # rope tables, mirroring the reference's f32 arithmetic
    inv_freq = (1.0 / (THETA ** (np.arange(0, RD, 2, dtype=f32) / f32(RD)))).astype(f32)
    pos = np.arange(offset, offset + T, dtype=f32)
    ang = (pos[:, None] * inv_freq[None, :]).astype(f32)     # [T, RD/2]
    ang = np.concatenate([ang, ang], axis=-1)                # [T, RD]
    cos_t = np.cos(ang).T                                    # [RD, T]
    sin_t = np.sin(ang).T
    cosT = np.concatenate([cos_t, cos_t], 0)                 # [128, T]
    sinT = np.concatenate([sin_t, sin_t], 0)
    # [128, NJ, 2, 512]: per-supertile contiguous cos+sin
    csT = np.stack(
        [cosT.reshape(128, NJ, 512), sinT.reshape(128, NJ, 512)], axis=2
    )

    # signed rotate-half permutation (2 heads per 128 partitions), as lhsT
    M = np.zeros((RD, RD), f32)
    for i in range(RD // 2):
        M[i, i + RD // 2] = -1.0
        M[i + RD // 2, i] = 1.0
    perm128 = np.zeros((128, 128), f32)
    perm128[:64, :64] = M
    perm128[64:, 64:] = M
    perm_lhsT = perm128.T

    # diagonal causal masks: block r masked where (128 r + p) > f
    p_idx = np.arange(128)[:, None]
    f_idx = np.arange(512)[None, :]
    masks = np.stack(
        [(128 * r + p_idx <= f_idx).astype(f32) for r in range(4)], axis=1
    )  # [128, 4, 512]

    kvw = np.broadcast_to(kv_norm_w[None, :], (128, LAT)).astype(f32)

    wuk_full = W_UK.reshape(H, HD, LAT)
    wuv_full = W_UV.reshape(H, HD, LAT)

    in_maps = []
    for b in range(2):
        xTb = _part_major(x[b].T)                            # [128, DC, T]
        # [128, NJ, DC//4, 4, 512]: per-(supertile, dc-quad) contiguous
        xTb = bf(
            xTb.reshape(128, DC // 4, 4, NJ, 512).transpose(0, 3, 1, 2, 4)
        )
        for hg in range(4):
            hs = slice(HPC * hg * HD, HPC * (hg + 1) * HD)          # content rows
            rs = slice(D_OUT + HPC * hg * RD, D_OUT + HPC * (hg + 1) * RD)
            heads = slice(HPC * hg, HPC * (hg + 1))
            wuk_c = wuk_full[heads]                                  # [4,128,512]
            wuv_c = wuv_full[heads]
            in_maps.append(
                {
                    "xT": xTb,
                    "wdkvT": bf(_part_major(W_DKV.T)),
                    "wkrT": bf(_part_major(W_KR[HPC * hg * RD : HPC * (hg + 1) * RD].T)),
                    "wqcT": bf(_part_major(W_Q[hs].T)),
                    "wqrT": bf(_part_major(W_Q[rs].T)),
                    "wuk": bf(wuk_c.transpose(1, 0, 2)),             # [128,4,512]
                    "wuvT": bf(
                        wuv_c.transpose(0, 2, 1)                     # [4,512,128]
                        .reshape(HPC, LC, 128, HD)
                        .transpose(2, 0, 1, 3)                       # [128,4,4,128]
                    ),
                    "owT": bf(
                        out_w[:, hs].T.reshape(HPC, 128, D_OUT).transpose(1, 0, 2)
                    ),
                    "kvw": bf(kvw),
                    "csT": bf(csT),
                    "perm": bf(perm_lhsT),
                    "masks": bf(masks),
                }
            )
    return in_maps


_NC_CACHE = {}


def get_nc(T=2048):
    if T not in _NC_CACHE:
        _NC_CACHE[T] = build_mla_nc(T)
    return _NC_CACHE[T]


LAST_RESULTS = None


def kernel(x, W_DKV, kv_norm_w, W_KR, W_Q, W_UK, W_UV, out_w, out_b, offset):
    global LAST_RESULTS
    import os

    x = np.asarray(x, np.float32)
    B, T, _ = x.shape
    nc = get_nc(T)
    in_maps = make_in_maps(
        x, W_DKV, kv_norm_w, W_KR, W_Q, W_UK, W_UV, out_w, offset, T
    )
    trace = os.environ.get("MLA_TRACE", "0") == "1"
    res = run_bass_kernel_spmd(
        nc, in_maps, core_ids=list(range(8)), trace=trace
    )
    LAST_RESULTS = res
    out = np.zeros((B, T, D_OUT), np.float32)
    for c, r in enumerate(res.results):
        out[c // 4] += np.asarray(r["out_p"], np.float32)
    out += np.asarray(out_b, np.float32)[None, None, :]
    return out


# revision 32
# speedup vs baseline: 1.0129x; 1.0085x over previous
"""DeepSeekV3 latent attention (MLA) Trainium2 Bass kernel.

Sharding: 8 cores = 2 batches x 4 head-groups (4 heads each).
Each core computes, for its (batch b, head-group hg):
  - c_kv = RMSNorm(x_b @ W_DKV.T) * w        (replicated across the 4 hg cores)
  - k_rope / q projections for its 4 heads (weights sliced on the head axis)
  - causal latent attention (no-max-sub softmax, exp/sum form)
  - out_partial = ctx_hg @ out_w[:, hg_cols].T   (row-parallel partial)
Host sums the 4 partials per batch and adds the bias.

Optimizations over the 738us baseline (trace-driven):
  - Softmax denominator no longer uses 160 ones-row matmuls: exp tiles are
    accumulated on DVE (bf16 adds) and ONE all-ones [128,128] matmul per
    context sums across partitions AND broadcasts in a single shot. The
    freed PSUM bank raises the score-bank rotation to 4.
  - Rope K tiles are zero-padded to full 128 rows (per-head tile, opposite
    half zeroed) so the rope LDWEIGHTS is a normal full-row load: the old
    64-row row_grp load could not overlap in-flight matmuls and cost
    ~200ns x 160 in double LDW serialization.
  - reciprocal_approx_fast (5x faster than reciprocal) reads the broadcast
    PSUM directly: kills the 3.4us PSUM-bank hostage + DVE FIFO clog at
    every context boundary and at the kernel tail.
  - qa for context i+1 is emitted before context i's last PV quad, so its
    PSUM waits and copies are fully covered; qa/cl drain copies alternate
    scalar/vector so neither FIFO gates the PE.
  - x / cos+sin DRAM layouts are contiguous-per-supertile (4KB descriptors)
    and x loads ride the idle sync queue: first matmul and HAM warmup come
    ~8us earlier.
  - wuk/wuvT/masks live in a whole-kernel pool loaded at the top of the
    gpsimd queue: phase B no longer waits on phase A pool teardown (which
    cost a 4.7us gap plus a HAM re-throttle at the seam).
  - Output DMA is one trigger per 128-token tile (4KB rows) instead of 4:
    the serialized trigger chain was most of the 5.7us tail.
Device layout: feature-on-partition, token-on-free throughout, so scores
come out as S^T [tk, tq] and probs feed the PV matmul with no transposes.
"""

import numpy as np
import ml_dtypes

import concourse.bass as bass
import concourse.tile as tile
from concourse import bacc
from concourse import mybir
from concourse.bass import ts
from concourse.bass_utils import run_bass_kernel_spmd

BF16 = mybir.dt.bfloat16
F32 = mybir.dt.float32
NPBF16 = ml_dtypes.bfloat16

H, HD, RD, LAT = 16, 128, 64, 512
D_IN = 2048
D_OUT = H * HD
HPC = 4  # heads per core
LC = LAT // 128
EPS = 1e-6
THETA = 10000.0
SCALE = 1.0 / float(np.sqrt(np.float32(HD + RD)))
AF = mybir.ActivationFunctionType
ALU = mybir.AluOpType


def build_mla_nc(T=2048):
    nc = bacc.Bacc("TRN2", target_bir_lowering=False)
    DC = D_IN // 128      # 16 contraction chunks for the x projections
    NT = T // 128         # 128-token tiles
    NJ = T // 512         # 512-token query supertiles

    # ---------------- I/O (all layouts are host-prepared, partition-major) ---
    xT = nc.dram_tensor("xT", [128, NJ, DC // 4, 4, 512], BF16, kind="ExternalInput")
    wdkvT = nc.dram_tensor("wdkvT", [128, DC, LAT], BF16, kind="ExternalInput")
    wkrT = nc.dram_tensor("wkrT", [128, DC, HPC * RD], BF16, kind="ExternalInput")
    wqcT = nc.dram_tensor("wqcT", [128, DC, HPC * HD], BF16, kind="ExternalInput")
    wqrT = nc.dram_tensor("wqrT", [128, DC, HPC * RD], BF16, kind="ExternalInput")
    wuk = nc.dram_tensor("wuk", [128, HPC, LAT], BF16, kind="ExternalInput")
    wuvT = nc.dram_tensor("wuvT", [128, HPC, LC, HD], BF16, kind="ExternalInput")
    owT = nc.dram_tensor("owT", [128, HPC, D_OUT], BF16, kind="ExternalInput")
    kvw = nc.dram_tensor("kvw", [128, LAT], BF16, kind="ExternalInput")
    csT = nc.dram_tensor("csT", [128, NJ, 2, 512], BF16, kind="ExternalInput")
    perm = nc.dram_tensor("perm", [128, 128], BF16, kind="ExternalInput")
    masks = nc.dram_tensor("masks", [128, 4, 512], BF16, kind="ExternalInput")
    out_p = nc.dram_tensor("out_p", [T, D_OUT], BF16, kind="ExternalOutput")

    with tile.TileContext(nc) as tc:
        with (
            tc.tile_pool(name="persist", bufs=1) as persist,
            tc.tile_pool(name="bw", bufs=1) as bw,
        ):
            # persistent activations
            ckv_nat = persist.tile([128, NT, LAT], BF16)   # [t%128, ttile, lat]
            ckvT = persist.tile([128, NT, LC, 128], BF16)  # [lat%128, ttile, lc, tok]
            # per-head rope K, zero-padded on the opposite 64-row half so the
            # attention-phase LDWEIGHTS is a normal full-row load
            kz = persist.tile([128, HPC, T], BF16)
            qTrot = persist.tile([128, 2, T], BF16)        # [pairrow, h//2, t]
            qcT = persist.tile([128, HPC, T], BF16)        # [hd, h, t]
            ctxT = persist.tile([128, HPC, T], BF16)       # [hd, h, t]

            # attention-phase weights, loaded at the very top of the gpsimd
            # queue so the A->B seam never waits on them
            wuk_s = bw.tile([128, HPC, LAT], BF16)
            wuvT_s = bw.tile([128, HPC, LC, HD], BF16)
            masks_s = bw.tile([128, 4, 512], BF16)
            ones128 = bw.tile([128, 128], BF16)

            # zero the unused rope halves once; rope writes fill the rest
            nc.vector.memset(kz[:, :, :], 0.0)
            nc.vector.memset(ones128, 1.0)

            # ============== Phase A: projections + RMSNorm + RoPE ===========
            with (
                tc.tile_pool(name="ps_a", bufs=4, space="PSUM") as ps_a,
                tc.tile_pool(name="ps_ck", bufs=1, space="PSUM") as ps_ck,
                tc.tile_pool(name="aw", bufs=1) as aw,
                tc.tile_pool(name="xs", bufs=2) as xs,
                tc.tile_pool(name="cs", bufs=2) as cs,
                tc.tile_pool(name="wka", bufs=2) as wka,
            ):
                wdkvT_s = aw.tile([128, DC, LAT], BF16)
                wkrT_s = aw.tile([128, DC, HPC * RD], BF16)
                wqcT_s = aw.tile([128, DC, HPC * HD], BF16)
                wqrT_s = aw.tile([128, DC, HPC * RD], BF16)
                # tiny-but-critical tensors first: kvw gates the c_kv norm
                # chain (and thus ckvT) at ~33us, perm gates the first rope
                perm_s = aw.tile([128, 128], BF16)
                nc.gpsimd.dma_start(perm_s, perm[:, :])
                kvw_s = aw.tile([128, LAT], BF16)
                nc.gpsimd.dma_start(kvw_s, kvw[:, :])
                # graded wdkv chunks: the dc-outer c_kv stream starts on the
                # first tiny chunk and never outruns the arriving data
                for sl in (slice(0, 1), slice(1, 2), slice(2, 4),
                           slice(4, 8), slice(8, 12), slice(12, 16)):
                    nc.gpsimd.dma_start(wdkvT_s[:, sl, :], wdkvT[:, sl, :])
                # q/k projection weights ride a second queue (scalar), in
                # consumption order (wkr -> wqr), so the proj groups never
                # stall on the single gpsimd queue; wqc goes on the sync
                # queue (emitted after the jt0 x triggers below) so each of
                # the three queues carries ~2MB in the first 35us
                for d4 in range(DC // 4):
                    sl = slice(4 * d4, 4 * d4 + 4)
                    nc.scalar.dma_start(wkrT_s[:, sl, :], wkrT[:, sl, :])
                for d4 in range(DC // 4):
                    sl = slice(4 * d4, 4 * d4 + 4)
                    nc.scalar.dma_start(wqrT_s[:, sl, :], wqrT[:, sl, :])
                eps_s = aw.tile([128, 1], F32)
                nc.vector.memset(eps_s, EPS)
                # attention weights ride the same queue, behind the A weights
                nc.gpsimd.dma_start(wuk_s, wuk[:, :, :])
                nc.gpsimd.dma_start(wuvT_s, wuvT[:, :, :, :])
                nc.gpsimd.dma_start(masks_s, masks[:, :, :])

                def rope_pair(raw, is_k, rc, jt, cos_s, sin_s, tag):
                    # raw: [128,512] sbuf with 2 heads' raw rope rows.
                    psr = ps_a.tile([128, 512], F32, tag="mm")
                    nc.tensor.matmul(psr, lhsT=perm_s, rhs=raw, start=True, stop=True)
                    tmp = wka.tile([128, 512], BF16, tag=f"{tag}_cos")
                    nc.vector.tensor_mul(tmp, raw, cos_s)
                    tmp2 = wka.tile([128, 512], BF16, tag=f"{tag}_sin")
                    nc.vector.tensor_mul(tmp2, psr, sin_s)
                    if is_k:
                        # per-head zero-padded tiles: head 2rc keeps the top
                        # 64 rows, head 2rc+1 the bottom 64 (rest stays 0)
                        nc.vector.tensor_add(
                            kz[0:64, 2 * rc, ts(jt, 512)], tmp[0:64, :], tmp2[0:64, :]
                        )
                        nc.vector.tensor_add(
                            kz[64:128, 2 * rc + 1, ts(jt, 512)],
                            tmp[64:128, :], tmp2[64:128, :],
                        )
                    else:
                        nc.vector.tensor_add(qTrot[:, rc, ts(jt, 512)], tmp, tmp2)

                rope_pending = []

                def flush_rope():
                    while rope_pending:
                        rope_pair(*rope_pending.pop(0))

                def load_x(jt, first=False):
                    xts = xs.tile([128, DC, 512], BF16, tag="x")
                    if first:
                        # fine-grained first chunks: first matmul starts as
                        # soon as dc0 lands
                        nc.sync.dma_start(xts[:, 0:1, :], xT[:, 0, 0, 0:1, :])
                        nc.sync.dma_start(xts[:, 1:2, :], xT[:, 0, 0, 1:2, :])
                        nc.sync.dma_start(xts[:, 2:4, :], xT[:, 0, 0, 2:4, :])
                    else:
                        nc.sync.dma_start(xts[:, 0:4, :], xT[:, jt, 0, :, :])
                    for q4 in range(1, 4):
                        nc.sync.dma_start(
                            xts[:, 4 * q4 : 4 * q4 + 4, :],
                            xT[:, jt, q4, :, :],
                        )
                    cs_t = cs.tile([128, 2, 512], BF16, tag="cs")
                    nc.scalar.dma_start(cs_t, csT[:, jt, :, :])
                    return xts, cs_t

                nxt = load_x(0, first=True)
                # wqc rides the sync queue behind the jt0 x tiles: needed by
                # the first q-content proj group at ~36us
                for d4 in range(DC // 4):
                    sl = slice(4 * d4, 4 * d4 + 4)
                    nc.sync.dma_start(wqcT_s[:, sl, :], wqcT[:, sl, :])
                for jt in range(NJ):
                    xts, cs_t = nxt
                    if jt + 1 < NJ:
                        nxt = load_x(jt + 1)
                    cos_s = cs_t[:, 0, :]
                    sin_s = cs_t[:, 1, :]

                    # --- c_kv + RMSNorm, dc-outer so each arriving x chunk
                    #     immediately unblocks 4 matmuls (smooth startup) ---
                    ps_c = ps_ck.tile([128, 4, 512], F32, tag="ckv")
                    for dc in range(DC):
                        for tt4 in range(4):
                            nc.tensor.matmul(
                                ps_c[:, tt4, :],
                                lhsT=xts[:, dc, ts(tt4, 128)],
                                rhs=wdkvT_s[:, dc, :],
                                start=(dc == 0),
                                stop=(dc == DC - 1),
                            )
                    for tt4 in range(4):
                        tt = jt * 4 + tt4
                        # sq lives in SBUF: a PSUM sq output would make the
                        # next proj group's bank wait on the norm chain
                        sq = wka.tile([128, LAT], BF16, tag="sq")
                        ssum = wka.tile([128, 1], F32, tag="ssum")
                        nc.scalar.activation(
                            sq, ps_c[:, tt4, :], AF.Square, accum_out=ssum
                        )
                        rstd = wka.tile([128, 1], F32, tag="rstd")
                        nc.scalar.activation(
                            rstd, ssum, AF.Sqrt, bias=eps_s, scale=1.0 / LAT
                        )
                        nc.vector.reciprocal(rstd, rstd)
                        nc.vector.scalar_tensor_tensor(
                            ckv_nat[:, tt, :], ps_c[:, tt4, :], rstd, kvw_s,
                            op0=ALU.mult, op1=ALU.mult,
                        )
                        # transposed copy for the QK side (single xbar
                        # transpose per token tile; contiguous destination)
                        nc.sync.dma_start_transpose(
                            ckvT[:, tt, :, :], ckv_nat[:, tt, :]
                        )

                    # --- rope + q projections, rc0 (heads 0/1) first so the
                    #     woven attention contexts unblock as early as possible
                    def proj_group(w_s, col):
                        ps = ps_a.tile([128, 512], F32, tag="mm")
                        for dc in range(DC):
                            nc.tensor.matmul(
                                ps,
                                lhsT=w_s[:, dc, ts(col, 128)],
                                rhs=xts[:, dc, :],
                                start=(dc == 0),
                                stop=(dc == DC - 1),
                            )
                        return ps

                    for rc in range(2):
                        ps = proj_group(wkrT_s, rc)
                        raw = wka.tile([128, 512], BF16, tag="k_raw")
                        nc.scalar.copy(raw, ps)
                        rope_pending.append((raw, True, rc, jt, cos_s, sin_s, "k"))
                        ps = proj_group(wqrT_s, rc)
                        raw = wka.tile([128, 512], BF16, tag="q_raw")
                        nc.scalar.copy(raw, ps)
                        rope_pending.append((raw, False, rc, jt, cos_s, sin_s, "q"))
                        for fc in (range(2) if rc == 0 else range(2, HPC)):
                            ps = proj_group(wqcT_s, fc)
                            nc.scalar.copy(qcT[:, fc, ts(jt, 512)], ps)
                            flush_rope()
                flush_rope()

            # ============== Phase B: attention, j outer / head inner ========
            with (
                tc.tile_pool(name="ps_s", bufs=4, space="PSUM") as ps_s,
                tc.tile_pool(name="ps_pv", bufs=1, space="PSUM") as ps_pv,
                tc.tile_pool(name="bw2", bufs=1) as bw2,
                tc.tile_pool(name="qa", bufs=3) as qa_pool,
                tc.tile_pool(name="exps", bufs=12) as exps,
                tc.tile_pool(name="wkb", bufs=3) as wkb,
                tc.tile_pool(name="dnp", bufs=2) as dnp,
                tc.tile_pool(name="outs", bufs=2) as outs,
            ):
                owT_s = bw2.tile([128, HPC, D_OUT], BF16)
                for hc4 in range(HPC):
                    nc.gpsimd.dma_start(owT_s[:, hc4, :], owT[:, hc4, :])

                def emit_qa(j, h):
                    qa_t = qa_pool.tile([128, LC, 512], BF16, tag="qa")
                    for lc in range(LC):
                        ps = ps_s.tile([128, 512], F32, tag="sc")
                        nc.tensor.matmul(
                            ps,
                            lhsT=wuk_s[:, h, ts(lc, 128)],
                            rhs=qcT[:, h, ts(j, 512)],
                            start=True,
                            stop=True,
                        )
                        # alternate engines so neither FIFO gates the copies
                        if lc % 2 == 0:
                            nc.vector.tensor_copy(qa_t[:, lc, :], ps)
                        else:
                            nc.scalar.copy(qa_t[:, lc, :], ps)
                    return qa_t

                def emit_qk_quad(j, h, quad, qa_t, dn_acc):
                    # diagonal quad: key tile tq only sees query columns
                    # >= 128*tq, so every matmul shrinks to N = 512-128*tq.
                    # Earlier quads initialize the full PSUM width, so the
                    # partial-width accumulation is safe.
                    diag = quad == j
                    exs = []
                    for tq in range(4):
                        tk = 4 * quad + tq
                        c0 = 128 * tq if diag else 0
                        n = 512 - c0
                        ps = ps_s.tile([128, 512], F32, tag="sc")
                        for lc in range(LC):
                            nc.tensor.matmul(
                                ps[:, c0:512],
                                lhsT=ckvT[:, tk, lc, :],
                                rhs=qa_t[:, lc, c0:512],
                                start=(lc == 0),
                                stop=False,
                            )
                        nc.tensor.matmul(
                            ps[:, c0:512],
                            lhsT=kz[:, h, ts(tk, 128)],
                            rhs=qTrot[:, h // 2, 512 * j + c0 : 512 * (j + 1)],
                            start=False,
                            stop=True,
                        )
                        ex = exps.tile([128, 512], BF16, tag="exp")
                        nc.scalar.activation(ex[:, 0:n], ps[:, c0:512], AF.Exp, scale=SCALE)
                        if diag:
                            # only the leading 128 columns straddle the
                            # diagonal; they share one triangular mask
                            nc.vector.tensor_mul(
                                ex[:, 0:128], ex[:, 0:128], masks_s[:, 0, 0:128]
                            )
                        # softmax denominator: accumulate exp tiles on DVE
                        # (replaces a ones-row matmul per tile)
                        if quad == 0 and tq == 0:
                            nc.vector.tensor_copy(dn_acc, ex)
                        else:
                            nc.vector.tensor_add(
                                dn_acc[:, c0:512], dn_acc[:, c0:512], ex[:, 0:n]
                            )
                        exs.append((ex, c0))
                    return exs

                def emit_pv_quad(j, quad, exs, ps_ctx):
                    ntk = 4 * (j + 1)
                    for tq in range(4):
                        tk = 4 * quad + tq
                        ex, c0 = exs[tq]
                        for lc in range(LC):
                            nc.tensor.matmul(
                                ps_ctx[:, lc, c0:512],
                                lhsT=ckv_nat[:, tk, ts(lc, 128)],
                                rhs=ex[:, 0 : 512 - c0],
                                start=(tk == 0),
                                stop=(tk == ntk - 1),
                            )

                def emit_dn_bcast(dn_acc):
                    # one matmul sums the 128 partition-partials AND
                    # broadcasts the result across all partitions
                    ps_bc = ps_s.tile([128, 512], F32, tag="sc")
                    nc.tensor.matmul(
                        ps_bc, lhsT=ones128, rhs=dn_acc, start=True, stop=True
                    )
                    return ps_bc

                def emit_drain_casts(ps_ctx):
                    cl = wkb.tile([128, LC, 512], BF16, tag="ctxlat")
                    for lc in range(LC):
                        if lc % 2 == 0:
                            nc.scalar.copy(cl[:, lc, :], ps_ctx[:, lc, :])
                        else:
                            nc.vector.tensor_copy(cl[:, lc, :], ps_ctx[:, lc, :])
                    return cl

                def emit_drain_rest(j, h, cl, ps_bc):
                    db = wkb.tile([128, 512], F32, tag="db")
                    nc.vector.reciprocal_approx_fast(db, ps_bc[:, :])
                    ps_uv = ps_s.tile([128, 512], F32, tag="sc")
                    for lc in range(LC):
                        nc.tensor.matmul(
                            ps_uv,
                            lhsT=wuvT_s[:, h, lc, :],
                            rhs=cl[:, lc, :],
                            start=(lc == 0),
                            stop=(lc == LC - 1),
                        )
                    nc.vector.tensor_mul(ctxT[:, h, ts(j, 512)], ps_uv, db)

                def emit_out_proj(j):
                    for tt4 in range(4):
                        tt = 4 * j + tt4
                        ot = outs.tile([128, D_OUT], BF16, tag="ot")
                        for oc in range(D_OUT // 512):
                            ps = ps_s.tile([128, 512], F32, tag="sc")
                            for hc in range(HPC):
                                nc.tensor.matmul(
                                    ps,
                                    lhsT=ctxT[:, hc, ts(tt, 128)],
                                    rhs=owT_s[:, hc, ts(oc, 512)],
                                    start=(hc == 0),
                                    stop=(hc == HPC - 1),
                                )
                            if oc % 2 == 0:
                                nc.scalar.copy(ot[:, ts(oc, 512)], ps)
                            else:
                                nc.vector.tensor_copy(ot[:, ts(oc, 512)], ps)
                        # one DMA per 128-token tile (4KB rows): the old
                        # per-chunk triggers serialized on the sync engine
                        nc.sync.dma_start(out_p[ts(tt, 128), :], ot[:, :])

                # software-pipelined emission: drains of context i-1 ride
                # behind context i's first QK quad; qa for context i+1 is
                # emitted before context i's last PV quad.
                # j1 before j0: j0's shallow contexts then sit where j2's
                # deep quads cover their drains
                j_order = [1, 0] + list(range(2, NJ))
                contexts = [(j, h) for j in j_order for h in range(HPC)]
                pending = None       # (j, h, ps_ctx, dn_acc)
                pending_out = None
                next_qa = emit_qa(*contexts[0])
                for idx, (j, h) in enumerate(contexts):
                    qa_t = next_qa
                    next_qa = None
                    cl = None
                    if pending is not None:
                        cl = emit_drain_casts(pending[2])
                    dn_acc = dnp.tile([128, 512], BF16, tag="dn")
                    exs = emit_qk_quad(j, h, 0, qa_t, dn_acc)
                    if pending is not None:
                        ps_bc = emit_dn_bcast(pending[3])
                    if pending_out is not None:
                        emit_out_proj(pending_out)
                        pending_out = None
                    if pending is not None:
                        emit_drain_rest(pending[0], pending[1], cl, ps_bc)
                        if pending[1] == HPC - 1:
                            pending_out = pending[0]
                        pending = None
                    ps_ctx = ps_pv.tile([128, LC, 512], F32, tag="pv")
                    for quad in range(j + 1):
                        if quad > 0:
                            exs = emit_qk_quad(j, h, quad, qa_t, dn_acc)
                        if quad == j and idx + 1 < len(contexts):
                            next_qa = emit_qa(*contexts[idx + 1])
                        emit_pv_quad(j, quad, exs, ps_ctx)
                    pending = (j, h, ps_ctx, dn_acc)
                cl = emit_drain_casts(pending[2])
                ps_bc = emit_dn_bcast(pending[3])
                emit_drain_rest(pending[0], pending[1], cl, ps_bc)
                emit_out_proj(NJ - 1)

    nc.finalize()
    return nc


def _part_major(a2d):
    """[R, C] -> [128, R//128, C] with partition = R % 128."""
    r, c = a2d.shape
    return np.ascontiguousarray(
        a2d.reshape(r // 128, 128, c).transpose(1, 0, 2)
    )


def make_in_maps(x, W_DKV, kv_norm_w, W_KR, W_Q, W_UK, W_UV, out_w, offset, T):
    """Host-side sharding/layout prep. Returns the 8 per-core input dicts."""
    f32 = np.float32
    x = np.asarray(x, f32)
    W_DKV = np.asarray(W_DKV, f32)
    kv_norm_w = np.asarray(kv_norm_w, f32)
    W_KR = np.asarray(W_KR, f32)
    W_Q = np.asarray(W_Q, f32)
    W_UK = np.asarray(W_UK, f32)
    W_UV = np.asarray(W_UV, f32)
    out_w = np.asarray(out_w, f32)
    offset = int(np.asarray(offset))
    DC = D_IN // 128
    NJ = T // 512

    def bf(a):
        return np.ascontiguousarray(a).astype(NPBF16)

    # rope tables, mirroring the reference's f32 arithmetic
    inv_freq = (1.0 / (THETA ** (np.arange(0, RD, 2, dtype=f32) / f32(RD)))).astype(f32)
    pos = np.arange(offset, offset + T, dtype=f32)
    ang = (pos[:, None] * inv_freq[None, :]).astype(f32)     # [T, RD/2]
    ang = np.concatenate([ang, ang], axis=-1)                # [T, RD]
    cos_t = np.cos(ang).T                                    # [RD, T]
    sin_t = np.sin(ang).T
    cosT = np.concatenate([cos_t, cos_t], 0)                 # [128, T]
    sinT = np.concatenate([sin_t, sin_t], 0)
    # [128, NJ, 2, 512]: per-supertile contiguous cos+sin
    csT = np.stack(
        [cosT.reshape(128, NJ, 512), sinT.reshape(128, NJ, 512)], axis=2
    )

    # signed rotate-half permutation (2 heads per 128 partitions), as lhsT
    M = np.zeros((RD, RD), f32)
    for i in range(RD // 2):
        M[i, i + RD // 2] = -1.0
        M[i + RD // 2, i] = 1.0
    perm128 = np.zeros((128, 128), f32)
    perm128[:64, :64] = M
    perm128[64:, 64:] = M
    perm_lhsT = perm128.T

    # diagonal causal masks: block r masked where (128 r + p) > f
    p_idx = np.arange(128)[:, None]
    f_idx = np.arange(512)[None, :]
    masks = np.stack(
        [(128 * r + p_idx <= f_idx).astype(f32) for r in range(4)], axis=1
    )  # [128, 4, 512]

    kvw = np.broadcast_to(kv_norm_w[None, :], (128, LAT)).astype(f32)

    wuk_full = W_UK.reshape(H, HD, LAT)
    wuv_full = W_UV.reshape(H, HD, LAT)

    in_maps = []
    for b in range(2):
        xTb = _part_major(x[b].T)                            # [128, DC, T]
        # [128, NJ, DC//4, 4, 512]: per-(supertile, dc-quad) contiguous
        xTb = bf(
            xTb.reshape(128, DC // 4, 4, NJ, 512).transpose(0, 3, 1, 2, 4)
        )
        for hg in range(4):
            hs = slice(HPC * hg * HD, HPC * (hg + 1) * HD)          # content rows
            rs = slice(D_OUT + HPC * hg * RD, D_OUT + HPC * (hg + 1) * RD)
            heads = slice(HPC * hg, HPC * (hg + 1))
            wuk_c = wuk_full[heads]                                  # [4,128,512]
            wuv_c = wuv_full[heads]
            in_maps.append(
                {
                    "xT": xTb,
                    "wdkvT": bf(_part_major(W_DKV.T)),
                    "wkrT": bf(_part_major(W_KR[HPC * hg * RD : HPC * (hg + 1) * RD].T)),
                    "wqcT": bf(_part_major(W_Q[hs].T)),
                    "wqrT": bf(_part_major(W_Q[rs].T)),
                    "wuk": bf(wuk_c.transpose(1, 0, 2)),             # [128,4,512]
                    "wuvT": bf(
                        wuv_c.transpose(0, 2, 1)                     # [4,512,128]
                        .reshape(HPC, LC, 128, HD)
                        .transpose(2, 0, 1, 3)                       # [128,4,4,128]
                    ),
                    "owT": bf(
                        out_w[:, hs].T.reshape(HPC, 128, D_OUT).transpose(1, 0, 2)
                    ),
                    "kvw": bf(kvw),
                    "csT": bf(csT),
                    "perm": bf(perm_lhsT),
                    "masks": bf(masks),
                }
            )
    return in_maps


_NC_CACHE = {}


def get_nc(T=2048):
    if T not in _NC_CACHE:
        _NC_CACHE[T] = build_mla_nc(T)
    return _NC_CACHE[T]


LAST_RESULTS = None


def kernel(x, W_DKV, kv_norm_w, W_KR, W_Q, W_UK, W_UV, out_w, out_b, offset):
    global LAST_RESULTS
    import os

    x = np.asarray(x, np.float32)
    B, T, _ = x.shape
    nc = get_nc(T)
    in_maps = make_in_maps(
        x, W_DKV, kv_norm_w, W_KR, W_Q, W_UK, W_UV, out_w, offset, T
    )
    trace = os.environ.get("MLA_TRACE", "0") == "1"
    res = run_bass_kernel_spmd(
        nc, in_maps, core_ids=list(range(8)), trace=trace
    )
    LAST_RESULTS = res
    out = np.zeros((B, T, D_OUT), np.float32)
    for c, r in enumerate(res.results):
        out[c // 4] += np.asarray(r["out_p"], np.float32)
    out += np.asarray(out_b, np.float32)[None, None, :]
    return out
